# revision 62
# baseline (speedup 1.0000x reference)
"""Causal self-attention on 8 TRN2 NeuronCores.

Sharding: core c handles batch b = c//2 and head-group g = c%2 (8 of 16 heads).
Each core computes its partial y^T = w_proj[slice].T @ o^T (contraction over its
512 o-channels); the host sums the two partials per batch and adds b_proj.

v2: host pre-transposes x and casts x/w to bf16 (all matmuls bf16, no PE
transposes), o strips stay in SBUF, reciprocal_approx_fast for softmax denom,
attention starts as early as possible with qkv emission spread into it.

Shapes (hardcoded): B=4, T=2048, C=1024, H=16, HD=64.
"""

import numpy as np

B, T, C, H = 4, 2048, 1024, 16
HD = C // H          # 64
G = 2                # head groups
NHL = H // G         # 8 heads per core
GQ = NHL * HD        # 512 channel slice per core
P = 128
NT = T // P          # 16 token tiles / k-chunks
NCHUNK = C // P      # 8 contraction chunks for qkv
SCALE = 1.0 / float(np.sqrt(HD))

_PROGRAM = None


def _emit(ctx, tc, aps, mybir, bass):
    nc = tc.nc
    f32 = mybir.dt.float32
    bf16 = mybir.dt.bfloat16
    EXP = mybir.ActivationFunctionType.Exp

    xT_d, wqkv_d, bqk_d, bv_d, wp_d, yT_d = (
        aps["xT"], aps["wqkv"], aps["bqk"], aps["bv"], aps["wp"], aps["yT"],
    )

    # ---------------- pools ----------------
    const = ctx.enter_context(tc.tile_pool(name="const", bufs=1))
    # psum: sc/proj 2x[128,1024] (4 banks) + pv windows 3x[128,512] (3 banks)
    # + dedicated qk/v emission bank 1x[128,512] so qkv emission never
    # contends with the attention score/window rings.
    import contextlib
    ps_main = ctx.enter_context(tc.tile_pool(name="ps_main", bufs=2, space="PSUM"))
    attn_psum = contextlib.ExitStack()
    ps_pv = attn_psum.enter_context(tc.tile_pool(name="ps_pv", bufs=3, space="PSUM"))
    ps_qv = attn_psum.enter_context(tc.tile_pool(name="ps_qv", bufs=1, space="PSUM"))

    xTp = ctx.enter_context(tc.tile_pool(name="xTp", bufs=1))
    qkp = ctx.enter_context(tc.tile_pool(name="qkp", bufs=8))
    vap = ctx.enter_context(tc.tile_pool(name="vap", bufs=16))
    ptp = ctx.enter_context(tc.tile_pool(name="ptp", bufs=7))
    otp = ctx.enter_context(tc.tile_pool(name="otp", bufs=1))
    rcp = ctx.enter_context(tc.tile_pool(name="rcp", bufs=3))
    wqkp = ctx.enter_context(tc.tile_pool(name="wqkp", bufs=4))
    wvp = ctx.enter_context(tc.tile_pool(name="wvp", bufs=1))
    wpp = ctx.enter_context(tc.tile_pool(name="wpp", bufs=1))
    ysp = ctx.enter_context(tc.tile_pool(name="ysp", bufs=3))

    # constants / weights: DMA issue order = arrival priority. The first
    # attention work needs wv + wqk{0,4} + the front halves of the xT
    # strips; wp is only needed at the very end.
    bqk_sb = const.tile([P, 8], f32)
    nc.sync.dma_start(bqk_sb[:], bqk_d[:])
    bvb = const.tile([P, GQ], f32)
    nc.sync.dma_start(bvb[:], bv_d[None, :].to_broadcast((P, GQ)))
    ones8 = const.tile([P, NHL, 1], bf16)
    nc.vector.memset(ones8[:], 1.0)

    # All weight/x DRAM tensors are pre-arranged by the host into the exact
    # device layout (partition-major), so every input DMA is 128 fat
    # contiguous lines instead of thousands of 1KB descriptors.
    wqk_tiles = {}

    def load_wqk_pair(j):
        if j not in wqk_tiles:
            w_t = wqkp.tile([P, NCHUNK, 2 * P], bf16, name=f"wqk_{j}",
                            tag="wqk")
            nc.sync.dma_start(
                w_t[:],
                wqkv_d[:, j * 2048:(j + 1) * 2048].rearrange(
                    "p (a n) -> p a n", a=NCHUNK))
            wqk_tiles[j] = w_t
        return wqk_tiles[j]

    load_wqk_pair(0)

    # x^T as [128 c-part, 4 token-quarters, 8 c-chunks, 512 t] — quarter q
    # is one contiguous 8KB line per partition.
    xT_all = xTp.tile([P, 4, NCHUNK, 512], bf16, name="xTall", tag="xT")
    for q in range(2):
        nc.sync.dma_start(
            xT_all[:, q, :, :],
            xT_d[:, q * 4096:(q + 1) * 4096].rearrange(
                "p (a n) -> p a n", a=NCHUNK))

    wv_t = wvp.tile([P, NCHUNK, GQ], bf16, name="wv", tag="wv")
    nc.sync.dma_start(
        wv_t[:],
        wqkv_d[:, 8192:12288].rearrange("p (a n) -> p a n", a=NCHUNK))
    load_wqk_pair(1)
    for q in range(2, 4):
        nc.sync.dma_start(
            xT_all[:, q, :, :],
            xT_d[:, q * 4096:(q + 1) * 4096].rearrange(
                "p (a n) -> p a n", a=NCHUNK))
    load_wqk_pair(2)
    load_wqk_pair(3)

    wp_t = wpp.tile([P, 4, C], bf16, name="wp", tag="wp")
    nc.sync.dma_start(wp_t[:], wp_d.rearrange("p (a n) -> p a n", a=4))

    def load_wp():
        pass

    # ---------------- qkv emission helpers ----------------
    qkT = []  # bf16 tiles [128 c', 2048 t]; 0..3 = qT, 4..7 = kT
    for ct in range(8):
        o_t = qkp.tile([P, T], bf16, name=f"qkT{ct}", tag="qkT")
        qkT.append(o_t)

    vaug = []  # [128 k, 8 heads, 65] per k-chunk (col 64 = ones for denom)
    for t in range(NT):
        va = vap.tile([P, NHL, HD + 1], bf16, name=f"vaug{t}", tag="vaug")
        nc.vector.tensor_copy(va[:, :, HD:HD + 1], ones8[:])
        vaug.append(va)

    def emit_qk_half(ct, twp, main=False):
        # one [128,1024] half of output tile ct (q cols twp*1024..).
        # main=True (pre-attention): one [128,1024] psum from the free main
        # ring; otherwise two 512-wide pieces through the dedicated ps_qv
        # bank so attention's score ring is untouched.
        j, qk = ct % 4, ct // 4
        w_t = load_wqk_pair(j)
        wsl = w_t[:, :, qk * P:(qk + 1) * P]
        if main:
            ps = ps_main.tile([P, 1024], f32, name=f"qkps_{ct}_{twp}",
                              tag="main")
            for sw in range(2):
                for a in range(NCHUNK):
                    nc.tensor.matmul(
                        ps[:, sw * 512:(sw + 1) * 512],
                        wsl[:, a, :],
                        xT_all[:, twp * 2 + sw, a, :],
                        start=(a == 0),
                        stop=(a == NCHUNK - 1),
                    )
            nc.vector.tensor_scalar_add(
                qkT[ct][:, twp * 1024:(twp + 1) * 1024], ps[:],
                bqk_sb[:, ct:ct + 1]
            )
            return
        for sw in range(2):
            emit_qk_piece(ct, twp, sw)

    def emit_v(t0, t1):
        for t in range(t0, t1):
            ps = ps_qv.tile([P, GQ], f32, name=f"vps_{t}", tag="ps_qv")
            for a in range(NCHUNK):
                nc.tensor.matmul(
                    ps[:],
                    xT_all[:, t // 4, a, (t % 4) * P:(t % 4 + 1) * P],
                    wv_t[:, a, :],
                    start=(a == 0),
                    stop=(a == NCHUNK - 1),
                )
            nc.vector.tensor_add(
                vaug[t][:, :, 0:HD],
                ps[:].rearrange("p (h d) -> p h d", h=NHL),
                bvb[:].rearrange("p (h d) -> p h d", h=NHL),
            )

    def v_fns(t0, t1):
        return [lambda t=t: emit_v(t, t + 1) for t in range(t0, t1)]

    def qk_fns(ct, twp):
        return [lambda sw=sw: emit_qk_piece(ct, twp, sw) for sw in range(2)]

    def emit_qk_piece(ct, twp, sw):
        j, qk = ct % 4, ct // 4
        w_t = load_wqk_pair(j)
        wsl = w_t[:, :, qk * P:(qk + 1) * P]
        ps = ps_qv.tile([P, 512], f32, name=f"qkps_{ct}_{twp}_{sw}",
                        tag="ps_qv")
        for a in range(NCHUNK):
            nc.tensor.matmul(
                ps[:],
                wsl[:, a, :],
                xT_all[:, twp * 2 + sw, a, :],
                start=(a == 0),
                stop=(a == NCHUNK - 1),
            )
        nc.vector.tensor_scalar_add(
            qkT[ct][:, twp * 1024 + sw * 512:twp * 1024 + (sw + 1) * 512],
            ps[:], bqk_sb[:, ct:ct + 1]
        )

    # ---------------- attention ----------------
    # Head pairs: head A on PE row strip 0, head B on strip 64; score pieces
    # for the two heads live in the two banks of one [128,1024] psum tile, so
    # the row-packed matmuls run concurrently and one exp covers both heads.
    # Quarter-outer loop: each 512-wide q-window accumulates PV fully, then
    # normalizes while the next window runs (pv pool rotation hides it).
    # o^T strips all live in one SBUF mega-tile [128, 16 windows, 512] so
    # proj can stream 1024-wide rhs slices.
    ot_mega = otp.tile([P, 16, 512], bf16, name="ot_mega", tag="ot")

    def attn_pair(hp, extra=None):
        qt = qkT[hp]
        kt = qkT[4 + hp]
        for m in range(4):  # quarter windows of 512 q
            # qkv-emission hooks are staggered between chunks so their
            # matmuls never sit in the PE queue ahead of a whole window's
            # scores (which would starve the scalar engine).
            fns = list(extra[m]) if extra and m in extra else []
            ws = m * 512
            pvt = {}
            for hh in range(2):
                pvt[hh] = ps_pv.tile(
                    [P, 512], f32, name=f"pv_{hp}_{m}_{hh}", tag="ps_pv")
            # diagonal chunks first: their affine_select/PV dependency chain
            # lands while the window pipeline is still filling, leaving the
            # long tail of full-width chunks dependency-light.
            order = list(range(4 * m, 4 * m + 4)) + list(range(4 * m))
            first, last = order[0], order[-1]
            pend_b = []
            for pos, i in enumerate(order):  # causal k-chunks for this window
                s = max(i * P, ws)
                o = s - ws
                # head A piece in cols [o, 512), head B in [512, 1024-o)
                sc = ps_main.tile([P, 1024], f32, name=f"sc_{hp}_{m}_{i}",
                                  tag="main")
                for hh in range(2):
                    r0 = hh * HD
                    c0 = o if hh == 0 else 512
                    nc.tensor.matmul(
                        sc[:, c0:c0 + 512 - o],
                        kt[r0:r0 + HD, i * P:(i + 1) * P],
                        qt[r0:r0 + HD, s:ws + 512],
                        start=True,
                        stop=True,
                    )
                pt = ptp.tile([P, 1024], bf16, name=f"pt_{hp}_{m}_{i}",
                              tag="pt")
                nc.scalar.activation(pt[:, o:1024 - o], sc[:, o:1024 - o],
                                     EXP, scale=SCALE)
                diag = i * P >= ws

                def emit_pv(i, o, pt, hh):
                    c0 = o if hh == 0 else 512
                    if i * P >= ws:
                        nc.gpsimd.affine_select(
                            out=pt[:, c0:c0 + P],
                            in_=pt[:, c0:c0 + P],
                            compare_op=mybir.AluOpType.is_ge,
                            fill=0.0,
                            base=0,
                            pattern=[[1, P]],
                            channel_multiplier=-1,
                        )
                    nc.tensor.matmul(
                        pvt[hh][0:HD + 1, o:],
                        vaug[i][:, 2 * hp + hh, :],
                        pt[:, c0:c0 + 512 - o],
                        start=(i == first),
                        stop=(i == last),
                    )

                emit_pv(i, o, pt, 0)
                # head B's PV trails two chunks: at the window boundary its
                # first matmul gates on the previous window's normalize
                # releasing a pv-ring slot, and the PE executes its queue in
                # order — emitted late, the gate is already open.
                pend_b.append((i, o, pt))
                if len(pend_b) > 2:
                    emit_pv(*pend_b.pop(0), 1)
                if fns and pos >= 1:
                    fns.pop(0)()
            while fns:
                fns.pop(0)()
            while pend_b:
                emit_pv(*pend_b.pop(0), 1)
            # normalize both heads: ot rows 0:64 = head A, 64:128 = head B.
            # den copy on ACT (Copy is in every table set; ACT idles here),
            # per-head recip so head A's chain unblocks the pv ring early.
            ot = ot_mega[:, hp * 4 + m, :]
            den = rcp.tile([1, 1024], f32, name=f"den_{hp}_{m}", tag="den")
            rc = rcp.tile([1, 1024], f32, name=f"rc_{hp}_{m}", tag="rc")
            for hh in range(2):
                nc.vector.tensor_copy(
                    den[:, hh * 512:(hh + 1) * 512], pvt[hh][HD:HD + 1, :])
                nc.vector.reciprocal_approx_fast(
                    rc[:, hh * 512:(hh + 1) * 512],
                    den[:, hh * 512:(hh + 1) * 512])
                rcb = rcp.tile([HD, 512], f32, name=f"rcb_{hp}_{m}_{hh}",
                               tag="rcb")
                nc.gpsimd.partition_broadcast(
                    rcb[:], rc[:, hh * 512:(hh + 1) * 512])
                nc.vector.tensor_mul(
                    ot[hh * HD:(hh + 1) * HD, :], pvt[hh][0:HD, :], rcb[:])

    # emission order: start attention pair 0 as soon as its first-window
    # inputs exist; spread the remaining qkv matmuls into the attention
    # quarters (dependencies are tracked by the tile framework; emission
    # order just shapes the schedule so the PE stream stays dense).
    # window m=0 of pair 0 only needs the first 512 q/k columns and
    # vaug[0..3]; emit exactly that, then backfill the rest via hooks.
    emit_qk_piece(0, 0, 0)
    emit_qk_piece(4, 0, 0)
    emit_v(0, 4)
    attn_pair(0, extra={
        0: [lambda: emit_qk_piece(0, 0, 1), lambda: emit_qk_piece(4, 0, 1)]
            + v_fns(4, 8) + [load_wp],
        1: qk_fns(0, 1) + qk_fns(4, 1) + v_fns(8, 12),
        2: v_fns(12, 16) + qk_fns(1, 0),
        3: qk_fns(5, 0),
    })
    attn_pair(1, extra={
        1: qk_fns(1, 1) + qk_fns(5, 1),
        2: qk_fns(2, 0),
        3: qk_fns(6, 0),
    })
    attn_pair(2, extra={
        1: qk_fns(2, 1) + qk_fns(6, 1),
        2: qk_fns(3, 0),
        3: qk_fns(7, 0),
    })
    attn_pair(3, extra={
        1: qk_fns(3, 1) + qk_fns(7, 1),
    })

    # ---------------- proj ----------------
    # attention's psum pools are dead now; reuse their banks for a second
    # proj ring so the 16 proj groups pipeline 4 deep instead of 2.
    attn_psum.close()
    ps_proj = ctx.enter_context(tc.tile_pool(name="ps_proj", bufs=2, space="PSUM"))
    for mt in range(NCHUNK):  # cout tiles
        for twp in range(2):
            pool = ps_main if (mt * 2 + twp) % 2 == 0 else ps_proj
            tag = "main" if pool is ps_main else "proj"
            ps = pool.tile([P, 1024], f32, name=f"yps_{mt}_{twp}", tag=tag)
            for a in range(4):
                for sw in range(2):
                    nc.tensor.matmul(
                        ps[:, sw * 512:(sw + 1) * 512],
                        wp_t[:, a, mt * P:(mt + 1) * P],
                        ot_mega[:, a * 4 + 2 * twp + sw, :],
                        start=(a == 0),
                        stop=(a == 3),
                    )
            ys = ysp.tile([P, 1024], f32, name=f"ys_{mt}_{twp}", tag="ys")
            nc.vector.tensor_copy(ys[:], ps[:])
            nc.sync.dma_start(
                yT_d[mt * P:(mt + 1) * P, twp * 1024:(twp + 1) * 1024], ys[:]
            )


def _build_program():
    import contextlib

    import concourse.bass as bass
    import concourse.mybir as mybir
    import concourse.tile as tile
    from concourse import bacc

    nc = bacc.Bacc("TRN2", target_bir_lowering=False, debug=False, num_devices=8)
    f32 = mybir.dt.float32
    bf16 = mybir.dt.bfloat16
    aps = {
        "xT": nc.dram_tensor("xT", [P, 4 * NCHUNK * 512], bf16,
                             kind="ExternalInput").ap(),
        "wqkv": nc.dram_tensor("wqkv", [P, NCHUNK * 3 * GQ], bf16,
                               kind="ExternalInput").ap(),
        "bqk": nc.dram_tensor("bqk", [P, 8], f32, kind="ExternalInput").ap(),
        "bv": nc.dram_tensor("bv", [GQ], f32, kind="ExternalInput").ap(),
        "wp": nc.dram_tensor("wp", [P, 4 * C], bf16, kind="ExternalInput").ap(),
        "yT": nc.dram_tensor("yT", [C, T], f32, kind="ExternalOutput").ap(),
    }
    with tile.TileContext(nc) as tc:
        with contextlib.ExitStack() as ctx:
            _emit(ctx, tc, aps, mybir, bass)
    nc.compile()
    return nc


def get_program():
    global _PROGRAM
    if _PROGRAM is None:
        _PROGRAM = _build_program()
    return _PROGRAM


def make_in_maps(x, w_qkv, b_qkv, w_proj):
    import ml_dtypes

    bf16 = ml_dtypes.bfloat16
    x = np.asarray(x, np.float32)
    w_qkv = np.asarray(w_qkv, np.float32)
    b_qkv = np.asarray(b_qkv, np.float32)
    w_proj = np.asarray(w_proj, np.float32)
    # x^T in device layout [128 c-part, 4 t-quarter, 8 c-chunk, 512 t]
    xT_b = []
    for b in range(B):
        xh = x[b].astype(bf16).T  # [C, T]
        xd = xh.reshape(NCHUNK, P, 4, 512).transpose(1, 2, 0, 3)
        xT_b.append(np.ascontiguousarray(xd.reshape(P, 4 * NCHUNK * 512)))
    in_maps = []
    for c in range(8):
        b = c // 2
        g = c % 2
        q0 = g * GQ
        wq = w_qkv[:, q0:q0 + GQ]
        wk = w_qkv[:, C + q0:C + q0 + GQ]
        wv = w_qkv[:, 2 * C + q0:2 * C + q0 + GQ]
        # device layout: 4 [q_j | k_j] 256-col pair blocks then wv, each
        # group partition-major [128, 8 chunks, cols] flattened
        wint = np.concatenate(
            [np.concatenate([wq[:, j * P:(j + 1) * P],
                             wk[:, j * P:(j + 1) * P]], axis=1)
             for j in range(4)] + [wv], axis=1).astype(bf16)
        w3 = wint.reshape(NCHUNK, P, 3 * GQ).transpose(1, 0, 2)  # [p, a, n]
        groups = [w3[:, :, j * 256:(j + 1) * 256].reshape(P, NCHUNK * 256)
                  for j in range(4)]
        groups.append(w3[:, :, 1024:1536].reshape(P, NCHUNK * 512))
        wqkv = np.ascontiguousarray(np.concatenate(groups, axis=1))
        bq = b_qkv[q0:q0 + GQ]
        bk = b_qkv[C + q0:C + q0 + GQ]
        bqk = np.ascontiguousarray(np.concatenate([bq, bk]).reshape(8, P).T)
        bv = np.ascontiguousarray(b_qkv[2 * C + q0:2 * C + q0 + GQ])
        wp4 = w_proj[q0:q0 + GQ, :].astype(bf16).reshape(4, P, C)
        wp = np.ascontiguousarray(
            wp4.transpose(1, 0, 2).reshape(P, 4 * C))
        in_maps.append({
            "xT": xT_b[b],
            "wqkv": wqkv,
            "bqk": bqk,
            "bv": bv,
            "wp": wp,
        })
    return in_maps


def combine_outputs(outs, b_proj):
    b_proj = np.asarray(b_proj, np.float32)
    y = np.empty((B, T, C), np.float32)
    for b in range(B):
        acc = outs[2 * b] + outs[2 * b + 1]  # [C, T]
        y[b] = acc.T + b_proj
    return y


def kernel(x, w_qkv, b_qkv, w_proj, b_proj, _trace=False):
    from concourse import bass_utils

    nc = get_program()
    in_maps = make_in_maps(x, w_qkv, b_qkv, w_proj)
    res = bass_utils.run_bass_kernel_spmd(
        nc, in_maps, core_ids=list(range(8)), trace=_trace
    )
    outs = [r["yT"] for r in res.results]
    y = combine_outputs(outs, b_proj)
    if _trace:
        return y, res
    return y


# revision 63
# speedup vs baseline: 1.0116x; 1.0116x over previous
"""Causal self-attention on 8 TRN2 NeuronCores.

Sharding: core c handles batch b = c//2 and head-group g = c%2 (8 of 16 heads).
Each core computes its partial y^T = w_proj[slice].T @ o^T (contraction over its
512 o-channels); the host sums the two partials per batch and adds b_proj.

v2: host pre-transposes x and casts x/w to bf16 (all matmuls bf16, no PE
transposes), o strips stay in SBUF, reciprocal_approx_fast for softmax denom,
attention starts as early as possible with qkv emission spread into it.

Shapes (hardcoded): B=4, T=2048, C=1024, H=16, HD=64.
"""

import numpy as np

B, T, C, H = 4, 2048, 1024, 16
HD = C // H          # 64
G = 2                # head groups
NHL = H // G         # 8 heads per core
GQ = NHL * HD        # 512 channel slice per core
P = 128
NT = T // P          # 16 token tiles / k-chunks
NCHUNK = C // P      # 8 contraction chunks for qkv
SCALE = 1.0 / float(np.sqrt(HD))

_PROGRAM = None


def _emit(ctx, tc, aps, mybir, bass):
    nc = tc.nc
    f32 = mybir.dt.float32
    bf16 = mybir.dt.bfloat16
    EXP = mybir.ActivationFunctionType.Exp

    xT_d, wqkv_d, bqk_d, bv_d, wp_d, yT_d = (
        aps["xT"], aps["wqkv"], aps["bqk"], aps["bv"], aps["wp"], aps["yT"],
    )

    # ---------------- pools ----------------
    const = ctx.enter_context(tc.tile_pool(name="const", bufs=1))
    # psum: sc/proj 2x[128,1024] (4 banks) + pv windows 3x[128,512] (3 banks)
    # + dedicated qk/v emission bank 1x[128,512] so qkv emission never
    # contends with the attention score/window rings.
    import contextlib
    ps_main = ctx.enter_context(tc.tile_pool(name="ps_main", bufs=2, space="PSUM"))
    attn_psum = contextlib.ExitStack()
    ps_pv = attn_psum.enter_context(tc.tile_pool(name="ps_pv", bufs=3, space="PSUM"))
    ps_qv = attn_psum.enter_context(tc.tile_pool(name="ps_qv", bufs=1, space="PSUM"))

    xTp = ctx.enter_context(tc.tile_pool(name="xTp", bufs=1))
    qkp = ctx.enter_context(tc.tile_pool(name="qkp", bufs=8))
    vap = ctx.enter_context(tc.tile_pool(name="vap", bufs=16))
    ptp = ctx.enter_context(tc.tile_pool(name="ptp", bufs=7))
    otp = ctx.enter_context(tc.tile_pool(name="otp", bufs=1))
    rcp = ctx.enter_context(tc.tile_pool(name="rcp", bufs=3))
    wqkp = ctx.enter_context(tc.tile_pool(name="wqkp", bufs=4))
    wvp = ctx.enter_context(tc.tile_pool(name="wvp", bufs=1))
    wpp = ctx.enter_context(tc.tile_pool(name="wpp", bufs=1))
    ysp = ctx.enter_context(tc.tile_pool(name="ysp", bufs=3))

    # constants / weights: DMA issue order = arrival priority. The first
    # attention work needs wv + wqk{0,4} + the front halves of the xT
    # strips; wp is only needed at the very end.
    bqk_sb = const.tile([P, 8], f32)
    nc.sync.dma_start(bqk_sb[:], bqk_d[:])
    bvb = const.tile([P, GQ], f32)
    nc.sync.dma_start(bvb[:], bv_d[None, :].to_broadcast((P, GQ)))
    ones8 = const.tile([P, NHL, 1], bf16)
    nc.vector.memset(ones8[:], 1.0)

    # All weight/x DRAM tensors are pre-arranged by the host into the exact
    # device layout (partition-major), so every input DMA is 128 fat
    # contiguous lines instead of thousands of 1KB descriptors.
    wqk_tiles = {}

    def load_wqk_pair(j):
        if j not in wqk_tiles:
            w_t = wqkp.tile([P, NCHUNK, 2 * P], bf16, name=f"wqk_{j}",
                            tag="wqk")
            nc.sync.dma_start(
                w_t[:],
                wqkv_d[:, j * 2048:(j + 1) * 2048].rearrange(
                    "p (a n) -> p a n", a=NCHUNK))
            wqk_tiles[j] = w_t
        return wqk_tiles[j]

    load_wqk_pair(0)

    # x^T as [128 c-part, 4 token-quarters, 8 c-chunks, 512 t] — quarter q
    # is one contiguous 8KB line per partition.
    xT_all = xTp.tile([P, 4, NCHUNK, 512], bf16, name="xTall", tag="xT")
    for q in range(2):
        nc.sync.dma_start(
            xT_all[:, q, :, :],
            xT_d[:, q * 4096:(q + 1) * 4096].rearrange(
                "p (a n) -> p a n", a=NCHUNK))

    wv_t = wvp.tile([P, NCHUNK, GQ], bf16, name="wv", tag="wv")
    nc.sync.dma_start(
        wv_t[:],
        wqkv_d[:, 8192:12288].rearrange("p (a n) -> p a n", a=NCHUNK))
    load_wqk_pair(1)
    for q in range(2, 4):
        nc.sync.dma_start(
            xT_all[:, q, :, :],
            xT_d[:, q * 4096:(q + 1) * 4096].rearrange(
                "p (a n) -> p a n", a=NCHUNK))
    load_wqk_pair(2)
    load_wqk_pair(3)

    wp_t = wpp.tile([P, 4, C], bf16, name="wp", tag="wp")
    nc.sync.dma_start(wp_t[:], wp_d.rearrange("p (a n) -> p a n", a=4))

    def load_wp():
        pass

    # ---------------- qkv emission helpers ----------------
    qkT = []  # bf16 tiles [128 c', 2048 t]; 0..3 = qT, 4..7 = kT
    for ct in range(8):
        o_t = qkp.tile([P, T], bf16, name=f"qkT{ct}", tag="qkT")
        qkT.append(o_t)

    vaug = []  # [128 k, 8 heads, 65] per k-chunk (col 64 = ones for denom)
    for t in range(NT):
        va = vap.tile([P, NHL, HD + 1], bf16, name=f"vaug{t}", tag="vaug")
        nc.vector.tensor_copy(va[:, :, HD:HD + 1], ones8[:])
        vaug.append(va)

    def emit_qk_half(ct, twp, main=False):
        # one [128,1024] half of output tile ct (q cols twp*1024..).
        # main=True (pre-attention): one [128,1024] psum from the free main
        # ring; otherwise two 512-wide pieces through the dedicated ps_qv
        # bank so attention's score ring is untouched.
        j, qk = ct % 4, ct // 4
        w_t = load_wqk_pair(j)
        wsl = w_t[:, :, qk * P:(qk + 1) * P]
        if main:
            ps = ps_main.tile([P, 1024], f32, name=f"qkps_{ct}_{twp}",
                              tag="main")
            for sw in range(2):
                for a in range(NCHUNK):
                    nc.tensor.matmul(
                        ps[:, sw * 512:(sw + 1) * 512],
                        wsl[:, a, :],
                        xT_all[:, twp * 2 + sw, a, :],
                        start=(a == 0),
                        stop=(a == NCHUNK - 1),
                    )
            nc.vector.tensor_scalar_add(
                qkT[ct][:, twp * 1024:(twp + 1) * 1024], ps[:],
                bqk_sb[:, ct:ct + 1]
            )
            return
        for sw in range(2):
            emit_qk_piece(ct, twp, sw)

    def emit_v(t0, t1):
        for t in range(t0, t1):
            ps = ps_qv.tile([P, GQ], f32, name=f"vps_{t}", tag="ps_qv")
            for a in range(NCHUNK):
                nc.tensor.matmul(
                    ps[:],
                    xT_all[:, t // 4, a, (t % 4) * P:(t % 4 + 1) * P],
                    wv_t[:, a, :],
                    start=(a == 0),
                    stop=(a == NCHUNK - 1),
                )
            nc.vector.tensor_add(
                vaug[t][:, :, 0:HD],
                ps[:].rearrange("p (h d) -> p h d", h=NHL),
                bvb[:].rearrange("p (h d) -> p h d", h=NHL),
            )

    def v_fns(t0, t1):
        return [lambda t=t: emit_v(t, t + 1) for t in range(t0, t1)]

    def qk_fns(ct, twp):
        return [lambda sw=sw: emit_qk_piece(ct, twp, sw) for sw in range(2)]

    def emit_qk_piece(ct, twp, sw):
        j, qk = ct % 4, ct // 4
        w_t = load_wqk_pair(j)
        wsl = w_t[:, :, qk * P:(qk + 1) * P]
        ps = ps_qv.tile([P, 512], f32, name=f"qkps_{ct}_{twp}_{sw}",
                        tag="ps_qv")
        for a in range(NCHUNK):
            nc.tensor.matmul(
                ps[:],
                wsl[:, a, :],
                xT_all[:, twp * 2 + sw, a, :],
                start=(a == 0),
                stop=(a == NCHUNK - 1),
            )
        nc.vector.tensor_scalar_add(
            qkT[ct][:, twp * 1024 + sw * 512:twp * 1024 + (sw + 1) * 512],
            ps[:], bqk_sb[:, ct:ct + 1]
        )

    # ---------------- attention ----------------
    # Head pairs: head A on PE row strip 0, head B on strip 64; score pieces
    # for the two heads live in the two banks of one [128,1024] psum tile, so
    # the row-packed matmuls run concurrently and one exp covers both heads.
    # Quarter-outer loop: each 512-wide q-window accumulates PV fully, then
    # normalizes while the next window runs (pv pool rotation hides it).
    # o^T strips all live in one SBUF mega-tile [128, 16 windows, 512] so
    # proj can stream 1024-wide rhs slices.
    ot_mega = otp.tile([P, 16, 512], bf16, name="ot_mega", tag="ot")

    def attn_pair(hp, extra=None):
        qt = qkT[hp]
        kt = qkT[4 + hp]
        for m in range(4):  # quarter windows of 512 q
            # qkv-emission hooks are staggered between chunks so their
            # matmuls never sit in the PE queue ahead of a whole window's
            # scores (which would starve the scalar engine).
            fns = list(extra[m]) if extra and m in extra else []
            ws = m * 512
            pvt = {}
            for hh in range(2):
                pvt[hh] = ps_pv.tile(
                    [P, 512], f32, name=f"pv_{hp}_{m}_{hh}", tag="ps_pv")
            # diagonal chunks first: their affine_select/PV dependency chain
            # lands while the window pipeline is still filling, leaving the
            # long tail of full-width chunks dependency-light.
            order = list(range(4 * m, 4 * m + 4)) + list(range(4 * m))
            first, last = order[0], order[-1]
            pend_b = []
            for pos, i in enumerate(order):  # causal k-chunks for this window
                s = max(i * P, ws)
                o = s - ws
                # head A piece in cols [o, 512), head B in [512, 1024-o)
                sc = ps_main.tile([P, 1024], f32, name=f"sc_{hp}_{m}_{i}",
                                  tag="main")
                for hh in range(2):
                    r0 = hh * HD
                    c0 = o if hh == 0 else 512
                    nc.tensor.matmul(
                        sc[:, c0:c0 + 512 - o],
                        kt[r0:r0 + HD, i * P:(i + 1) * P],
                        qt[r0:r0 + HD, s:ws + 512],
                        start=True,
                        stop=True,
                    )
                pt = ptp.tile([P, 1024], bf16, name=f"pt_{hp}_{m}_{i}",
                              tag="pt")
                nc.scalar.activation(pt[:, o:1024 - o], sc[:, o:1024 - o],
                                     EXP, scale=SCALE)
                diag = i * P >= ws

                def emit_pv(i, o, pt, hh):
                    c0 = o if hh == 0 else 512
                    if i * P >= ws:
                        nc.gpsimd.affine_select(
                            out=pt[:, c0:c0 + P],
                            in_=pt[:, c0:c0 + P],
                            compare_op=mybir.AluOpType.is_ge,
                            fill=0.0,
                            base=0,
                            pattern=[[1, P]],
                            channel_multiplier=-1,
                        )
                    nc.tensor.matmul(
                        pvt[hh][0:HD + 1, o:],
                        vaug[i][:, 2 * hp + hh, :],
                        pt[:, c0:c0 + 512 - o],
                        start=(i == first),
                        stop=(i == last),
                    )

                emit_pv(i, o, pt, 0)
                # head B's PV trails two chunks: at the window boundary its
                # first matmul gates on the previous window's normalize
                # releasing a pv-ring slot, and the PE executes its queue in
                # order — emitted late, the gate is already open.
                pend_b.append((i, o, pt))
                if len(pend_b) > 2:
                    emit_pv(*pend_b.pop(0), 1)
                if fns and pos >= 1:
                    fns.pop(0)()
            while fns:
                fns.pop(0)()
            while pend_b:
                emit_pv(*pend_b.pop(0), 1)
            # normalize both heads: ot rows 0:64 = head A, 64:128 = head B.
            # den copy on ACT (Copy is in every table set; ACT idles here),
            # per-head recip so head A's chain unblocks the pv ring early.
            ot = ot_mega[:, hp * 4 + m, :]
            den = rcp.tile([1, 1024], f32, name=f"den_{hp}_{m}", tag="den")
            rc = rcp.tile([1, 1024], f32, name=f"rc_{hp}_{m}", tag="rc")
            for hh in range(2):
                nc.vector.tensor_copy(
                    den[:, hh * 512:(hh + 1) * 512], pvt[hh][HD:HD + 1, :])
                nc.vector.reciprocal_approx_fast(
                    rc[:, hh * 512:(hh + 1) * 512],
                    den[:, hh * 512:(hh + 1) * 512])
                rcb = rcp.tile([HD, 512], f32, name=f"rcb_{hp}_{m}_{hh}",
                               tag="rcb")
                nc.gpsimd.partition_broadcast(
                    rcb[:], rc[:, hh * 512:(hh + 1) * 512])
                nc.vector.tensor_mul(
                    ot[hh * HD:(hh + 1) * HD, :], pvt[hh][0:HD, :], rcb[:])

    # emission order: start attention pair 0 as soon as its first-window
    # inputs exist; spread the remaining qkv matmuls into the attention
    # quarters (dependencies are tracked by the tile framework; emission
    # order just shapes the schedule so the PE stream stays dense).
    emit_qk_half(0, 0, main=True)
    emit_v(0, 2)
    emit_qk_half(4, 0, main=True)
    emit_v(2, 4)
    attn_pair(0, extra={
        0: v_fns(4, 8) + [load_wp],
        1: qk_fns(0, 1) + qk_fns(4, 1) + v_fns(8, 12),
        2: v_fns(12, 16) + qk_fns(1, 0),
        3: qk_fns(5, 0),
    })
    attn_pair(1, extra={
        1: qk_fns(1, 1) + qk_fns(5, 1),
        2: qk_fns(2, 0),
        3: qk_fns(6, 0),
    })
    attn_pair(2, extra={
        1: qk_fns(2, 1) + qk_fns(6, 1),
        2: qk_fns(3, 0),
        3: qk_fns(7, 0),
    })
    attn_pair(3, extra={
        1: qk_fns(3, 1) + qk_fns(7, 1),
    })

    # ---------------- proj ----------------
    # attention's psum pools are dead now; reuse their banks for a second
    # proj ring so the 16 proj groups pipeline 4 deep instead of 2.
    attn_psum.close()
    ps_proj = ctx.enter_context(tc.tile_pool(name="ps_proj", bufs=2, space="PSUM"))
    for mt in range(NCHUNK):  # cout tiles
        for twp in range(2):
            pool = ps_main if (mt * 2 + twp) % 2 == 0 else ps_proj
            tag = "main" if pool is ps_main else "proj"
            ps = pool.tile([P, 1024], f32, name=f"yps_{mt}_{twp}", tag=tag)
            for a in range(4):
                for sw in range(2):
                    nc.tensor.matmul(
                        ps[:, sw * 512:(sw + 1) * 512],
                        wp_t[:, a, mt * P:(mt + 1) * P],
                        ot_mega[:, a * 4 + 2 * twp + sw, :],
                        start=(a == 0),
                        stop=(a == 3),
                    )
            ys = ysp.tile([P, 1024], f32, name=f"ys_{mt}_{twp}", tag="ys")
            nc.vector.tensor_copy(ys[:], ps[:])
            nc.sync.dma_start(
                yT_d[mt * P:(mt + 1) * P, twp * 1024:(twp + 1) * 1024], ys[:]
            )


def _build_program():
    import contextlib

    import concourse.bass as bass
    import concourse.mybir as mybir
    import concourse.tile as tile
    from concourse import bacc

    nc = bacc.Bacc("TRN2", target_bir_lowering=False, debug=False, num_devices=8)
    f32 = mybir.dt.float32
    bf16 = mybir.dt.bfloat16
    aps = {
        "xT": nc.dram_tensor("xT", [P, 4 * NCHUNK * 512], bf16,
                             kind="ExternalInput").ap(),
        "wqkv": nc.dram_tensor("wqkv", [P, NCHUNK * 3 * GQ], bf16,
                               kind="ExternalInput").ap(),
        "bqk": nc.dram_tensor("bqk", [P, 8], f32, kind="ExternalInput").ap(),
        "bv": nc.dram_tensor("bv", [GQ], f32, kind="ExternalInput").ap(),
        "wp": nc.dram_tensor("wp", [P, 4 * C], bf16, kind="ExternalInput").ap(),
        "yT": nc.dram_tensor("yT", [C, T], f32, kind="ExternalOutput").ap(),
    }
    with tile.TileContext(nc) as tc:
        with contextlib.ExitStack() as ctx:
            _emit(ctx, tc, aps, mybir, bass)
    nc.compile()
    return nc


def get_program():
    global _PROGRAM
    if _PROGRAM is None:
        _PROGRAM = _build_program()
    return _PROGRAM


def make_in_maps(x, w_qkv, b_qkv, w_proj):
    import ml_dtypes

    bf16 = ml_dtypes.bfloat16
    x = np.asarray(x, np.float32)
    w_qkv = np.asarray(w_qkv, np.float32)
    b_qkv = np.asarray(b_qkv, np.float32)
    w_proj = np.asarray(w_proj, np.float32)
    # x^T in device layout [128 c-part, 4 t-quarter, 8 c-chunk, 512 t]
    xT_b = []
    for b in range(B):
        xh = x[b].astype(bf16).T  # [C, T]
        xd = xh.reshape(NCHUNK, P, 4, 512).transpose(1, 2, 0, 3)
        xT_b.append(np.ascontiguousarray(xd.reshape(P, 4 * NCHUNK * 512)))
    in_maps = []
    for c in range(8):
        b = c // 2
        g = c % 2
        q0 = g * GQ
        wq = w_qkv[:, q0:q0 + GQ]
        wk = w_qkv[:, C + q0:C + q0 + GQ]
        wv = w_qkv[:, 2 * C + q0:2 * C + q0 + GQ]
        # device layout: 4 [q_j | k_j] 256-col pair blocks then wv, each
        # group partition-major [128, 8 chunks, cols] flattened
        wint = np.concatenate(
            [np.concatenate([wq[:, j * P:(j + 1) * P],
                             wk[:, j * P:(j + 1) * P]], axis=1)
             for j in range(4)] + [wv], axis=1).astype(bf16)
        w3 = wint.reshape(NCHUNK, P, 3 * GQ).transpose(1, 0, 2)  # [p, a, n]
        groups = [w3[:, :, j * 256:(j + 1) * 256].reshape(P, NCHUNK * 256)
                  for j in range(4)]
        groups.append(w3[:, :, 1024:1536].reshape(P, NCHUNK * 512))
        wqkv = np.ascontiguousarray(np.concatenate(groups, axis=1))
        bq = b_qkv[q0:q0 + GQ]
        bk = b_qkv[C + q0:C + q0 + GQ]
        bqk = np.ascontiguousarray(np.concatenate([bq, bk]).reshape(8, P).T)
        bv = np.ascontiguousarray(b_qkv[2 * C + q0:2 * C + q0 + GQ])
        wp4 = w_proj[q0:q0 + GQ, :].astype(bf16).reshape(4, P, C)
        wp = np.ascontiguousarray(
            wp4.transpose(1, 0, 2).reshape(P, 4 * C))
        in_maps.append({
            "xT": xT_b[b],
            "wqkv": wqkv,
            "bqk": bqk,
            "bv": bv,
            "wp": wp,
        })
    return in_maps


def combine_outputs(outs, b_proj):
    b_proj = np.asarray(b_proj, np.float32)
    y = np.empty((B, T, C), np.float32)
    for b in range(B):
        acc = outs[2 * b] + outs[2 * b + 1]  # [C, T]
        y[b] = acc.T + b_proj
    return y


def kernel(x, w_qkv, b_qkv, w_proj, b_proj, _trace=False):
    from concourse import bass_utils

    nc = get_program()
    in_maps = make_in_maps(x, w_qkv, b_qkv, w_proj)
    res = bass_utils.run_bass_kernel_spmd(
        nc, in_maps, core_ids=list(range(8)), trace=_trace
    )
    outs = [r["yT"] for r in res.results]
    y = combine_outputs(outs, b_proj)
    if _trace:
        return y, res
    return y
